# revision 14
# baseline (speedup 1.0000x reference)
"""Trainium2 Bass kernel for nn_DecoderTimeAttention.

Per (batch, node) pair (828 total, padded to 832 = 8 cores x 104 pairs,
2 pairs per kernel iteration):
  Q = Xq @ Wq * 0.25, K = Xk @ Wk, V = Xk @ Wv       (per-head dk=16, H=8)
  S_h = Q_h K_h^T          -> attn_score output (fp32)
  tv  = concat_h(S_h V_h) @ Wot ; time_value = LN(tv + Xq)
  ct  = concat_h(S_h T_h) @ Wtg ; cross_target = LN(ct)

Layout: head dims permuted (PI) so strip g (rows 32g..32g+31) holds head g
(j<16) and head g+4 (j>=16).  Scores: one K=32 matmul per strip with N=256
spanning masked K/Q variants for both pairs (off-diagonal garbage quadrants
are skipped by the copies).  ST comes out pair-on-partitions; value matmuls
are K=64 per (pair, strip) with combined (head, head+4) rhs; junk rows are
killed by zero rows in the Wot/Wtg A/B variants.  Residual added on the PE
via an identity matmul.  One PSUM bank per score strip (concurrent row-tiles
must not share a bank).
"""

import numpy as np
import ml_dtypes

import concourse.bass as bass
import concourse.bacc as bacc
import concourse.tile as tile
from concourse import mybir
from concourse.bass_utils import run_bass_kernel_spmd

F32 = mybir.dt.float32
BF16 = mybir.dt.bfloat16
AX = mybir.AluOpType
AF = mybir.ActivationFunctionType

N_CORES = 8
B, NN, L, D = 4, 207, 64, 128
H, DK = 8, 16
BN = B * NN              # 828
BN_PAD = 832             # 8 * 104
PAIRS_PER_CORE = BN_PAD // N_CORES   # 104
ITERS = PAIRS_PER_CORE // 2          # 52
LN_EPS = 1e-5

bf16 = ml_dtypes.bfloat16


def _perm():
    # pi[32g + j] = 16g + j (j<16) else 16(g+4) + (j-16)
    p = np.zeros(128, dtype=np.int64)
    for g in range(4):
        for j in range(32):
            p[32 * g + j] = 16 * g + j if j < 16 else 16 * (g + 4) + (j - 16)
    return p


PI = _perm()
MASK_TOP = np.array([1.0 if (i % 32) < 16 else 0.0 for i in range(128)],
                    dtype=np.float32)
MASK_BOT = 1.0 - MASK_TOP


def _weights_np(Wq, bq, Wk, bk, Wv, bv, Wot, bot, Wtg, btg,
                g_time, b_time, g_tgt, b_tgt):
    """Host-side weight preprocessing. Returns dict name->np array."""
    s = 0.25  # 1/sqrt(dk)
    wq_pi = (Wq * s)[:, PI]
    wk_pi = Wk[:, PI]
    wv_pi = Wv[:, PI]
    out = {
        "wq_pi": wq_pi,
        "wq_top": wq_pi * MASK_TOP[None, :],
        "wq_bot": wq_pi * MASK_BOT[None, :],
        "wk_pi": wk_pi,
        "wk_top": wk_pi * MASK_TOP[None, :],
        "wk_bot": wk_pi * MASK_BOT[None, :],
        "wv_pi": wv_pi,
        "wot_a": Wot[PI, :] * MASK_TOP[:, None],
        "wot_b": Wot[PI, :] * MASK_BOT[:, None],
        "wtg_a": Wtg[PI, :] * MASK_TOP[:, None],
        "wtg_b": Wtg[PI, :] * MASK_BOT[:, None],
        "ident": np.eye(128, dtype=np.float32),
    }
    out = {k: v.astype(bf16) for k, v in out.items()}
    # fp32 per-partition biases for the Q/K casts (zero-cost in tensor_scalar)
    bq_pi = (bq * s)[PI].astype(np.float32)
    bk_pi = bk[PI].astype(np.float32)
    out["bq_pi"] = (bq_pi)[:, None]
    out["bq_top"] = (bq_pi * MASK_TOP)[:, None]
    out["bq_bot"] = (bq_pi * MASK_BOT)[:, None]
    out["bk_pi"] = (bk_pi)[:, None]
    out["bk_top"] = (bk_pi * MASK_TOP)[:, None]
    out["bk_bot"] = (bk_pi * MASK_BOT)[:, None]
    # conditional extras (fp32 rows, DMA-broadcast to 128 partitions)
    out["bot_row"] = bot[None, :].astype(bf16)
    out["btg_row"] = btg[None, :].astype(bf16)
    out["bv_pi_row"] = bv[PI][None, :].astype(np.float32)
    out["g_time_row"] = g_time[None, :].astype(np.float32)
    out["b_time_row"] = b_time[None, :].astype(np.float32)
    out["g_tgt_row"] = g_tgt[None, :].astype(np.float32)
    out["b_tgt_row"] = b_tgt[None, :].astype(np.float32)
    return out


def build_bass(iters=ITERS, has_bv=False, has_bot=False, has_btg=False,
               gb_time=False, gb_tgt=False, stage=99, repeat=1):
    """Build the Bass module (one NeuronCore program, SPMD across 8)."""
    nc = bacc.Bacc("TRN2", target_bir_lowering=False, debug=False,
                   enable_asserts=False)

    xq = nc.dram_tensor("xq", [iters, 128, 128], BF16, kind="ExternalInput")
    xk = nc.dram_tensor("xk", [iters, 128, 128], BF16, kind="ExternalInput")
    xt = nc.dram_tensor("xt", [iters, 128, 128], BF16, kind="ExternalInput")
    wnames_bf = ["wq_pi", "wq_top", "wq_bot", "wk_pi", "wk_top", "wk_bot",
                 "wv_pi", "wot_a", "wot_b", "wtg_a", "wtg_b", "ident"]
    wd = {n: nc.dram_tensor(n, [128, 128], BF16, kind="ExternalInput")
          for n in wnames_bf}
    bnames = ["bq_pi", "bq_top", "bq_bot", "bk_pi", "bk_top", "bk_bot"]
    bd = {n: nc.dram_tensor(n, [128, 1], F32, kind="ExternalInput")
          for n in bnames}
    rows_bf = ["bot_row", "btg_row"]
    rd = {n: nc.dram_tensor(n, [1, 128], BF16, kind="ExternalInput")
          for n in rows_bf}
    rows_f32 = ["bv_pi_row", "g_time_row", "b_time_row", "g_tgt_row",
                "b_tgt_row"]
    gd = {n: nc.dram_tensor(n, [1, 128], F32, kind="ExternalInput")
          for n in rows_f32}

    s_out = nc.dram_tensor("s_out", [iters // 2, 128, 1024], F32,
                           kind="ExternalOutput")
    ot_out = nc.dram_tensor("ot_out", [iters // 4, 128, 512], F32,
                            kind="ExternalOutput")
    tg_out = nc.dram_tensor("tg_out", [iters // 4, 128, 512], F32,
                            kind="ExternalOutput")

    with tile.TileContext(nc) as tc:
        with (
            tc.tile_pool(name="consts", bufs=1) as consts,
            tc.tile_pool(name="inp", bufs=3) as inp,
            tc.tile_pool(name="mid", bufs=2) as mid,
            tc.tile_pool(name="big", bufs=2) as bigp,
            tc.tile_pool(name="outp", bufs=3) as outp,
            tc.tile_pool(name="stats", bufs=4) as statp,
            tc.tile_pool(name="pqk", bufs=1, space="PSUM") as pqkp,
            tc.tile_pool(name="psg", bufs=1, space="PSUM") as psgp,
            tc.tile_pool(name="ptv", bufs=1, space="PSUM") as ptvp,
        ):
            W = {}
            for n in wnames_bf:
                W[n] = consts.tile([128, 128], BF16, tag=n, name=n)
                nc.sync.dma_start(out=W[n], in_=wd[n][:, :])
            BIA = {}
            for n in bnames:
                BIA[n] = consts.tile([128, 1], F32, tag=n, name=n + "_sb")
                nc.sync.dma_start(out=BIA[n], in_=bd[n][:, :])
            ROW = {}
            for n in rows_bf:
                ROW[n] = consts.tile([1, 128], BF16, tag=n, name=n + "_sb")
                nc.sync.dma_start(out=ROW[n], in_=rd[n][:, :])
            for n in rows_f32:
                ROW[n] = consts.tile([128, 128], F32, tag=n, name=n + "_sb")
                nc.sync.dma_start(out=ROW[n],
                                  in_=gd[n][0:1, :].to_broadcast([128, 128]))
            eps_t = consts.tile([128, 1], F32, tag="eps")
            nc.vector.memset(eps_t, LN_EPS)
            ones_col = consts.tile([1, 128], BF16, tag="ones_col")
            nc.vector.memset(ones_col, 1.0)

            O4 = {}
            for ii in range(iters * repeat):
                i = ii % iters
                # ---- input loads: 4-iter blocks; xq/xk transposed ----
                if i % 4 == 0:
                    xqT4 = inp.tile([128, 512], BF16, tag="xqT4")
                    nc.sync.dma_start(
                        out=xqT4,
                        in_=xq[i:i + 4].rearrange("c p d -> (c p) d"),
                        transpose=True)
                    xkT4 = inp.tile([128, 512], BF16, tag="xkT4")
                    nc.sync.dma_start(
                        out=xkT4,
                        in_=xk[i:i + 4].rearrange("c p d -> (c p) d"),
                        transpose=True)
                    xtp4 = inp.tile([128, 4, 128], BF16, tag="xtp4")
                    nc.sync.dma_start(
                        out=xtp4, in_=xt[i:i + 4].rearrange("c p d -> p c d"))
                xqT = xqT4[:, 128 * (i % 4):128 * (i % 4) + 128]
                xkT = xkT4[:, 128 * (i % 4):128 * (i % 4) + 128]
                xtp = xtp4[:, i % 4, :]

                # ---- projections (PE) into pqk [128, 1024] (2 banks) ----
                # bank0: QT_pi@0 QT_top@128 QT_bot@256 KT_pi@384
                # bank1: KT_top@512 KT_bot@640 V@768 pot@896
                pqk = pqkp.tile([128, 1024], F32, tag="pqk")
                nc.tensor.matmul(pqk[:, 0:128], W["wq_pi"], xqT,
                                 tile_position=(0, 0))
                nc.tensor.matmul(pqk[:, 128:256], W["wq_top"], xqT,
                                 tile_position=(0, 0))
                nc.tensor.matmul(pqk[:, 256:384], W["wq_bot"], xqT,
                                 tile_position=(0, 0))
                nc.tensor.matmul(pqk[:, 384:512], W["wk_pi"], xkT,
                                 tile_position=(0, 0))
                nc.tensor.matmul(pqk[:, 512:640], W["wk_top"], xkT,
                                 tile_position=(0, 0))
                nc.tensor.matmul(pqk[:, 640:768], W["wk_bot"], xkT,
                                 tile_position=(0, 0))
                # V natural, both pairs in one matmul: Xk @ Wv_pi
                nc.tensor.matmul(pqk[:, 768:896], xkT, W["wv_pi"],
                                 tile_position=(0, 0))

                # ---- casts to SBUF bf16 (with Q/K biases folded) ----
                qtpi = mid.tile([128, 128], BF16, tag="qtpi")
                ktpi = mid.tile([128, 128], BF16, tag="ktpi")
                # qtab/ktab cols: [top-p0 | bot-p0 | top-p1 | bot-p1]
                qtab = mid.tile([128, 256], BF16, tag="qtab")
                ktab = mid.tile([128, 256], BF16, tag="ktab")
                qtab_v = qtab.rearrange("P (p v k) -> P p v k", p=2, v=2)
                ktab_v = ktab.rearrange("P (p v k) -> P p v k", p=2, v=2)
                nc.vector.tensor_scalar_add(qtpi, pqk[:, 0:128], BIA["bq_pi"])
                nc.scalar.activation(ktpi, pqk[:, 384:512], AF.Identity,
                                     bias=BIA["bk_pi"])
                nc.vector.tensor_scalar_add(
                    qtab_v[:, :, 0, :],
                    pqk[:, 128:256].rearrange("P (p k) -> P p k", p=2),
                    BIA["bq_top"])
                nc.scalar.activation(
                    qtab_v[:, :, 1, :],
                    pqk[:, 256:384].rearrange("P (p k) -> P p k", p=2),
                    AF.Identity, bias=BIA["bq_bot"])
                nc.vector.tensor_scalar_add(
                    ktab_v[:, :, 0, :],
                    pqk[:, 512:640].rearrange("P (p k) -> P p k", p=2),
                    BIA["bk_top"])
                nc.scalar.activation(
                    ktab_v[:, :, 1, :],
                    pqk[:, 640:768].rearrange("P (p k) -> P p k", p=2),
                    AF.Identity, bias=BIA["bk_bot"])
                v_sb = mid.tile([128, 128], BF16, tag="v_sb")
                if has_bv:
                    nc.vector.tensor_tensor(v_sb, pqk[:, 768:896],
                                            ROW["bv_pi_row"], AX.add)
                else:
                    nc.vector.tensor_copy(v_sb, pqk[:, 768:896])

                if stage < 2:
                    continue
                # ---- scores (PE): per strip g one S and one ST matmul ----
                # psg [128, 2048] (4 banks): per g: S@512g(256), ST@512g+256
                psg = psgp.tile([128, 2048], F32, tag="psg")
                for g in range(4):
                    st = slice(32 * g, 32 * g + 32)
                    nc.tensor.matmul(psg[:, 512 * g:512 * g + 256],
                                     qtpi[st, :], ktab[st, :],
                                     tile_position=(32 * g, 0))
                    nc.tensor.matmul(psg[:, 512 * g + 256:512 * g + 512],
                                     ktpi[st, :], qtab[st, :],
                                     tile_position=(32 * g, 0))

                if stage < 3:
                    continue
                # ---- S -> SBUF fp32; ST -> SBUF bf16 (valid quadrants) ----
                if i % 2 == 0:
                    s_sb2 = bigp.tile([128, 1024], F32, tag="s_sb2")
                s_sb = s_sb2[:, 512 * (i % 2):512 * (i % 2) + 512]
                st_sb = bigp.tile([128, 512], BF16, tag="st_sb")
                # s_sb col = 64h + k, h = 4b + g  ->  col = 256b + 64g + k
                s_v = s_sb.rearrange("P (b g k) -> P g b k", b=2, g=4)
                # psg S col = 512g + 128r + 64b + k (r = rhs pair half)
                psg_s = psg.rearrange("P (g q r b k) -> P g q r b k",
                                      g=4, q=2, r=2, b=2)
                nc.vector.tensor_copy(s_v[0:64], psg_s[0:64, :, 0, 0, :, :])
                nc.scalar.copy(s_v[64:128], psg_s[64:128, :, 0, 1, :, :])
                # st_sb col = 128g + c (c: [hg 64 | hg4 64]), rows (p, kt)
                st_v = st_sb.rearrange("P (g c) -> P g c", g=4)
                psg_t = psg.rearrange("P (g q r c) -> P g q r c",
                                      g=4, q=2, r=2)
                nc.vector.tensor_copy(st_v[0:64], psg_t[0:64, :, 1, 0, :])
                nc.scalar.copy(st_v[64:128], psg_t[64:128, :, 1, 1, :])

                if i % 2 == 1:
                    nc.sync.dma_start(out=s_out[i // 2], in_=s_sb2)

                if stage < 4:
                    continue
                # ---- value matmuls (PE): K=64 per (pair, strip) ----
                # ptv [128, 1024] (2 banks): TVCT_p0@0:256, ptg@256:384,
                #                            TVCT_p1@512:768
                ptv = ptvp.tile([128, 1024], F32, tag="ptv")
                for p in range(2):
                    pk = slice(64 * p, 64 * p + 64)
                    for g in range(4):
                        st = slice(32 * g, 32 * g + 32)
                        rhs = st_sb[pk, 128 * g:128 * g + 128]
                        nc.tensor.matmul(ptv[st, 512 * p:512 * p + 128],
                                         v_sb[pk, st], rhs,
                                         tile_position=(64 * p, 32 * g))
                        nc.tensor.matmul(ptv[st, 512 * p + 128:512 * p + 256],
                                         xtp[pk, st], rhs,
                                         tile_position=(64 * p, 32 * g))
                tvct = mid.tile([128, 512], BF16, tag="tvct")
                nc.vector.tensor_copy(tvct[:, 0:256], ptv[:, 0:256])
                nc.scalar.copy(tvct[:, 256:512], ptv[:, 512:768])

                if stage < 5:
                    continue
                # ---- output projections + residual (PE) ----
                pot = pqk[:, 896:1024]
                ptg = ptv[:, 256:384]
                for p in range(2):
                    tok = slice(64 * p, 64 * p + 64)
                    c = 256 * p
                    if has_bot:
                        nc.tensor.matmul(pot[tok, :], ones_col[:, 0:64],
                                         ROW["bot_row"], start=True,
                                         stop=False, tile_position=(0, 64 * p))
                    nc.tensor.matmul(pot[tok, :], tvct[:, c:c + 64],
                                     W["wot_a"], start=not has_bot,
                                     stop=False, tile_position=(0, 64 * p))
                    nc.tensor.matmul(pot[tok, :], tvct[:, c + 64:c + 128],
                                     W["wot_b"], start=False, stop=False,
                                     tile_position=(0, 64 * p))
                    nc.tensor.matmul(pot[tok, :], xqT[:, tok], W["ident"],
                                     start=False, stop=True,
                                     tile_position=(0, 64 * p))
                    if has_btg:
                        nc.tensor.matmul(ptg[tok, :], ones_col[:, 0:64],
                                         ROW["btg_row"], start=True,
                                         stop=False, tile_position=(0, 64 * p))
                    nc.tensor.matmul(ptg[tok, :], tvct[:, c + 128:c + 192],
                                     W["wtg_a"], start=not has_btg,
                                     stop=False, tile_position=(0, 64 * p))
                    nc.tensor.matmul(ptg[tok, :], tvct[:, c + 192:c + 256],
                                     W["wtg_b"], start=False, stop=True,
                                     tile_position=(0, 64 * p))

                if stage < 6:
                    continue
                # ---- LayerNorms ----
                for (src, outdram, gamma, beta, use_gb, on_act) in (
                        (pot, ot_out, "g_time_row", "b_time_row", gb_time,
                         True),
                        (ptg, tg_out, "g_tgt_row", "b_tgt_row", gb_tgt,
                         False)):
                    st6 = statp.tile([128, 6], F32, tag="st6")
                    mv = statp.tile([128, 2], F32, tag="mv")
                    rstd = statp.tile([128, 1], F32, tag="rstd")
                    nc.vector.bn_stats(st6, src[:, :])
                    nc.vector.bn_aggr(mv, st6)
                    nc.scalar.activation(rstd, mv[:, 1:2], AF.Sqrt,
                                         bias=eps_t)
                    nc.vector.reciprocal(rstd, rstd)
                    if i % 4 == 0:
                        o4 = outp.tile([128, 512], F32, tag="o4_" + gamma,
                                       name=f"o4_{gamma}_{ii}")
                        O4[gamma] = o4
                    o_sb = O4[gamma][:, 128 * (i % 4):128 * (i % 4) + 128]
                    if not use_gb:
                        if on_act:
                            nmu = statp.tile([128, 1], F32, tag="nmu")
                            nc.vector.tensor_scalar(nmu, mv[:, 0:1], rstd,
                                                    -1.0, AX.mult, AX.mult)
                            nc.scalar.activation(o_sb, src[:, :], AF.Identity,
                                                 bias=nmu, scale=rstd)
                        else:
                            nc.vector.tensor_scalar(o_sb, src[:, :],
                                                    mv[:, 0:1], rstd,
                                                    AX.subtract, AX.mult)
                    else:
                        z = outp.tile([128, 128], F32, tag="z_" + gamma)
                        nc.vector.scalar_tensor_tensor(
                            z, src[:, :], mv[:, 0:1], ROW[gamma],
                            AX.subtract, AX.mult)
                        nc.vector.scalar_tensor_tensor(
                            o_sb, z, rstd, ROW[beta],
                            AX.mult, AX.add)
                    if i % 4 == 3:
                        nc.sync.dma_start(out=outdram[i // 4], in_=O4[gamma])

    nc.compile()
    return nc


_BUILD_CACHE = {}


def _get_nc(key, **kw):
    if key not in _BUILD_CACHE:
        _BUILD_CACHE[key] = build_bass(**kw)
    return _BUILD_CACHE[key]


def kernel(time_features_Q, time_features_K, target_features_K,
           Wq, bq, Wk, bk, Wv, bv, Wot, bot, Wtg, btg,
           g_time, b_time, g_tgt, b_tgt):
    args = [np.asarray(a, dtype=np.float32) for a in
            (time_features_Q, time_features_K, target_features_K,
             Wq, bq, Wk, bk, Wv, bv, Wot, bot, Wtg, btg,
             g_time, b_time, g_tgt, b_tgt)]
    (Xq, Xk, Xt, Wq, bq, Wk, bk, Wv, bv, Wot, bot, Wtg, btg,
     g_time, b_time, g_tgt, b_tgt) = args

    wnp = _weights_np(Wq, bq, Wk, bk, Wv, bv, Wot, bot, Wtg, btg,
                      g_time, b_time, g_tgt, b_tgt)
    has_bv = bool(np.any(bv != 0))
    has_bot = bool(np.any(bot != 0))
    has_btg = bool(np.any(btg != 0))
    gb_time = bool(np.any(g_time != 1) or np.any(b_time != 0))
    gb_tgt = bool(np.any(g_tgt != 1) or np.any(b_tgt != 0))
    key = (ITERS, has_bv, has_bot, has_btg, gb_time, gb_tgt)
    nc = _get_nc(key, iters=ITERS, has_bv=has_bv, has_bot=has_bot,
                 has_btg=has_btg, gb_time=gb_time, gb_tgt=gb_tgt)

    def shard(x, perm=None):
        flat = x.reshape(BN, L, D)
        if perm is not None:
            flat = flat[:, :, perm]
        pad = np.concatenate([flat, np.zeros((BN_PAD - BN, L, D),
                                             np.float32)], axis=0)
        return pad.astype(bf16).reshape(N_CORES, ITERS, 128, D)

    xq_s, xk_s, xt_s = shard(Xq), shard(Xk), shard(Xt, perm=PI)

    in_maps = []
    for c in range(N_CORES):
        m = {"xq": np.ascontiguousarray(xq_s[c]),
             "xk": np.ascontiguousarray(xk_s[c]),
             "xt": np.ascontiguousarray(xt_s[c])}
        m.update({k: np.ascontiguousarray(v) for k, v in wnp.items()})
        in_maps.append(m)

    res = run_bass_kernel_spmd(nc, in_maps, core_ids=list(range(N_CORES)))
    outs = res.results

    def unstage_s(a):
        # [ITERS//2, 128, 1024] -> [pairs, h, q, k]
        a = a.reshape(ITERS // 2, 128, 2, 512).transpose(0, 2, 1, 3)
        return a.reshape(PAIRS_PER_CORE, 64, 8, 64).transpose(0, 2, 1, 3)

    def unstage_o(a):
        # [ITERS//4, 128, 512] -> [pairs, 64, 128]
        a = a.reshape(ITERS // 4, 128, 4, 128).transpose(0, 2, 1, 3)
        return a.reshape(PAIRS_PER_CORE, 64, 128)

    s_full = np.concatenate([unstage_s(r["s_out"]) for r in outs], axis=0)
    ot_full = np.concatenate([unstage_o(r["ot_out"]) for r in outs], axis=0)
    tg_full = np.concatenate([unstage_o(r["tg_out"]) for r in outs], axis=0)

    attn_score = s_full[:BN].reshape(B, NN, 8, 64, 64).astype(np.float32)
    time_value = ot_full[:BN].reshape(B, NN, 64, 128).astype(np.float32)
    cross_target = tg_full[:BN].reshape(B, NN, 64, 128).astype(np.float32)
    return attn_score, time_value, cross_target


# revision 17
# speedup vs baseline: 1.4465x; 1.4465x over previous
"""Trainium2 Bass kernel for nn_DecoderTimeAttention.

Per (batch, node) pair (828 total, padded to 832 = 8 cores x 104 pairs,
2 pairs per kernel iteration):
  Q = Xq @ Wq * 0.25, K = Xk @ Wk, V = Xk @ Wv       (per-head dk=16, H=8)
  S_h = Q_h K_h^T          -> attn_score output (fp32)
  tv  = concat_h(S_h V_h) @ Wot ; time_value = LN(tv + Xq)
  ct  = concat_h(S_h T_h) @ Wtg ; cross_target = LN(ct)

Layout: head dims permuted (PI) so strip g (rows 32g..32g+31) holds head g
(j<16) and head g+4 (j>=16).  Scores: one K=32 matmul per strip with N=256
spanning masked K/Q variants for both pairs (off-diagonal garbage quadrants
are skipped by the copies).  ST comes out pair-on-partitions; value matmuls
are K=64 per (pair, strip) with combined (head, head+4) rhs; junk rows are
killed by zero rows in the Wot/Wtg A/B variants.  Residual added on the PE
via an identity matmul.  One PSUM bank per score strip (concurrent row-tiles
must not share a bank).
"""

import numpy as np
import ml_dtypes

import concourse.bass as bass
import concourse.bacc as bacc
import concourse.tile as tile
from concourse import mybir
from concourse.bass_utils import run_bass_kernel_spmd

F32 = mybir.dt.float32
BF16 = mybir.dt.bfloat16
AX = mybir.AluOpType
AF = mybir.ActivationFunctionType

N_CORES = 8
B, NN, L, D = 4, 207, 64, 128
H, DK = 8, 16
BN = B * NN              # 828
BN_PAD = 832             # 8 * 104
PAIRS_PER_CORE = BN_PAD // N_CORES   # 104
ITERS = PAIRS_PER_CORE // 2          # 52
LN_EPS = 1e-5

bf16 = ml_dtypes.bfloat16


def _perm():
    # pi[32g + j] = 16g + j (j<16) else 16(g+4) + (j-16)
    p = np.zeros(128, dtype=np.int64)
    for g in range(4):
        for j in range(32):
            p[32 * g + j] = 16 * g + j if j < 16 else 16 * (g + 4) + (j - 16)
    return p


PI = _perm()
MASK_TOP = np.array([1.0 if (i % 32) < 16 else 0.0 for i in range(128)],
                    dtype=np.float32)
MASK_BOT = 1.0 - MASK_TOP


def _weights_np(Wq, bq, Wk, bk, Wv, bv, Wot, bot, Wtg, btg,
                g_time, b_time, g_tgt, b_tgt):
    """Host-side weight preprocessing. Returns dict name->np array."""
    s = 0.25  # 1/sqrt(dk)
    wq_pi = (Wq * s)[:, PI]
    wk_pi = Wk[:, PI]
    wv_pi = Wv[:, PI]
    out = {
        "wq_pi": wq_pi,
        "wq_top": wq_pi * MASK_TOP[None, :],
        "wq_bot": wq_pi * MASK_BOT[None, :],
        "wk_pi": wk_pi,
        "wk_top": wk_pi * MASK_TOP[None, :],
        "wk_bot": wk_pi * MASK_BOT[None, :],
        "wv_pi": wv_pi,
        "wot_a": Wot[PI, :] * MASK_TOP[:, None],
        "wot_b": Wot[PI, :] * MASK_BOT[:, None],
        "wtg_a": Wtg[PI, :] * MASK_TOP[:, None],
        "wtg_b": Wtg[PI, :] * MASK_BOT[:, None],
        "ident": np.eye(128, dtype=np.float32),
    }
    out = {k: v.astype(bf16) for k, v in out.items()}
    # fp32 per-partition biases for the Q/K casts (zero-cost in tensor_scalar)
    bq_pi = (bq * s)[PI].astype(np.float32)
    bk_pi = bk[PI].astype(np.float32)
    out["bq_pi"] = (bq_pi)[:, None]
    out["bq_top"] = (bq_pi * MASK_TOP)[:, None]
    out["bq_bot"] = (bq_pi * MASK_BOT)[:, None]
    out["bk_pi"] = (bk_pi)[:, None]
    out["bk_top"] = (bk_pi * MASK_TOP)[:, None]
    out["bk_bot"] = (bk_pi * MASK_BOT)[:, None]
    # conditional extras (fp32 rows, DMA-broadcast to 128 partitions)
    out["bot_row"] = bot[None, :].astype(bf16)
    out["btg_row"] = btg[None, :].astype(bf16)
    out["bv_pi_row"] = bv[PI][None, :].astype(np.float32)
    out["g_time_row"] = g_time[None, :].astype(np.float32)
    out["b_time_row"] = b_time[None, :].astype(np.float32)
    out["g_tgt_row"] = g_tgt[None, :].astype(np.float32)
    out["b_tgt_row"] = b_tgt[None, :].astype(np.float32)
    return out


def build_bass(iters=ITERS, has_bv=False, has_bot=False, has_btg=False,
               gb_time=False, gb_tgt=False, stage=99, repeat=1):
    """Build the Bass module (one NeuronCore program, SPMD across 8)."""
    nc = bacc.Bacc("TRN2", target_bir_lowering=False, debug=False,
                   enable_asserts=False)

    xq = nc.dram_tensor("xq", [iters, 128, 128], BF16, kind="ExternalInput")
    xk = nc.dram_tensor("xk", [iters, 128, 128], BF16, kind="ExternalInput")
    xt = nc.dram_tensor("xt", [iters, 128, 128], BF16, kind="ExternalInput")
    wnames_bf = ["wq_pi", "wq_top", "wq_bot", "wk_pi", "wk_top", "wk_bot",
                 "wv_pi", "wot_a", "wot_b", "wtg_a", "wtg_b", "ident"]
    wd = {n: nc.dram_tensor(n, [128, 128], BF16, kind="ExternalInput")
          for n in wnames_bf}
    bnames = ["bq_pi", "bq_top", "bq_bot", "bk_pi", "bk_top", "bk_bot"]
    bd = {n: nc.dram_tensor(n, [128, 1], F32, kind="ExternalInput")
          for n in bnames}
    rows_bf = ["bot_row", "btg_row"]
    rd = {n: nc.dram_tensor(n, [1, 128], BF16, kind="ExternalInput")
          for n in rows_bf}
    rows_f32 = ["bv_pi_row", "g_time_row", "b_time_row", "g_tgt_row",
                "b_tgt_row"]
    gd = {n: nc.dram_tensor(n, [1, 128], F32, kind="ExternalInput")
          for n in rows_f32}

    s_out = nc.dram_tensor("s_out", [iters // 4, 128, 2048], F32,
                           kind="ExternalOutput")
    ot_out = nc.dram_tensor("ot_out", [iters // 4, 128, 512], F32,
                            kind="ExternalOutput")
    tg_out = nc.dram_tensor("tg_out", [iters // 4, 128, 512], F32,
                            kind="ExternalOutput")

    with tile.TileContext(nc) as tc:
        with (
            tc.tile_pool(name="consts", bufs=1) as consts,
            tc.tile_pool(name="inp", bufs=3) as inp,
            tc.tile_pool(name="mid", bufs=2) as mid,
            tc.tile_pool(name="big", bufs=2) as bigp,
            tc.tile_pool(name="outp", bufs=3) as outp,
            tc.tile_pool(name="stats", bufs=4) as statp,
            tc.tile_pool(name="pqk", bufs=2, space="PSUM") as pqkp,
            tc.tile_pool(name="psg", bufs=1, space="PSUM") as psgp,
            tc.tile_pool(name="ptv", bufs=1, space="PSUM") as ptvp,
        ):
            W = {}
            for n in wnames_bf:
                W[n] = consts.tile([128, 128], BF16, tag=n, name=n)
                nc.sync.dma_start(out=W[n], in_=wd[n][:, :])
            BIA = {}
            for n in bnames:
                BIA[n] = consts.tile([128, 1], F32, tag=n, name=n + "_sb")
                nc.sync.dma_start(out=BIA[n], in_=bd[n][:, :])
            ROW = {}
            for n in rows_bf:
                ROW[n] = consts.tile([1, 128], BF16, tag=n, name=n + "_sb")
                nc.sync.dma_start(out=ROW[n], in_=rd[n][:, :])
            for n in rows_f32:
                ROW[n] = consts.tile([128, 128], F32, tag=n, name=n + "_sb")
                nc.sync.dma_start(out=ROW[n],
                                  in_=gd[n][0:1, :].to_broadcast([128, 128]))
            eps_t = consts.tile([128, 1], F32, tag="eps")
            nc.vector.memset(eps_t, LN_EPS)
            ones_col = consts.tile([1, 128], BF16, tag="ones_col")
            nc.vector.memset(ones_col, 1.0)

            O4 = {}
            for ii in range(iters * repeat):
                i = ii % iters
                # ---- input loads: 4-iter blocks; xq/xk transposed ----
                if i % 4 == 0:
                    xqT4 = inp.tile([128, 512], BF16, tag="xqT4")
                    nc.sync.dma_start(
                        out=xqT4,
                        in_=xq[i:i + 4].rearrange("c p d -> (c p) d"),
                        transpose=True)
                    xkT4 = inp.tile([128, 512], BF16, tag="xkT4")
                    nc.sync.dma_start(
                        out=xkT4,
                        in_=xk[i:i + 4].rearrange("c p d -> (c p) d"),
                        transpose=True)
                    xtp4 = inp.tile([128, 4, 128], BF16, tag="xtp4")
                    nc.sync.dma_start(
                        out=xtp4, in_=xt[i:i + 4].rearrange("c p d -> p c d"))
                xqT = xqT4[:, 128 * (i % 4):128 * (i % 4) + 128]
                xkT = xkT4[:, 128 * (i % 4):128 * (i % 4) + 128]
                xtp = xtp4[:, i % 4, :]

                # ---- projections per 2-iter block (4 pairs, N=256) ----
                if i % 2 == 0:
                    u4 = 256 * ((i % 4) // 2)
                    xqT2 = xqT4[:, u4:u4 + 256]
                    xkT2 = xkT4[:, u4:u4 + 256]
                    qtpi2 = mid.tile([128, 256], BF16, tag="qtpi2")
                    ktpi2 = mid.tile([128, 256], BF16, tag="ktpi2")
                    # qtab2/ktab2 cols: 4 pairs x [top | bot] (64 each)
                    qtab2 = mid.tile([128, 512], BF16, tag="qtab2")
                    ktab2 = mid.tile([128, 512], BF16, tag="ktab2")
                    qtab2_v = qtab2.rearrange("P (p v k) -> P p v k",
                                              p=4, v=2)
                    ktab2_v = ktab2.rearrange("P (p v k) -> P p v k",
                                              p=4, v=2)
                    for (wname, rhs2, outt, bia, on_act) in (
                            ("wq_pi", xqT2, qtpi2, "bq_pi", False),
                            ("wk_pi", xkT2, ktpi2, "bk_pi", True),
                            ("wq_top", xqT2, qtab2_v[:, :, 0, :], "bq_top",
                             False),
                            ("wq_bot", xqT2, qtab2_v[:, :, 1, :], "bq_bot",
                             True),
                            ("wk_top", xkT2, ktab2_v[:, :, 0, :], "bk_top",
                             False),
                            ("wk_bot", xkT2, ktab2_v[:, :, 1, :], "bk_bot",
                             True)):
                        pp = pqkp.tile([128, 256], F32, tag="pp",
                                       name=f"pp_{wname}_{ii}")
                        nc.tensor.matmul(pp, W[wname], rhs2,
                                         tile_position=(0, 0))
                        src_v = pp.rearrange("P (p k) -> P p k", p=4)
                        if on_act:
                            nc.scalar.activation(outt, src_v if outt is not
                                                 qtpi2 and outt is not ktpi2
                                                 else pp, AF.Identity,
                                                 bias=BIA[bia])
                        else:
                            nc.vector.tensor_scalar_add(
                                outt, src_v if outt is not qtpi2 and
                                outt is not ktpi2 else pp, BIA[bia])
                u2 = 128 * (i % 2)
                qtpi = qtpi2[:, u2:u2 + 128]
                ktpi = ktpi2[:, u2:u2 + 128]
                qtab = qtab2[:, 2 * u2:2 * u2 + 256]
                ktab = ktab2[:, 2 * u2:2 * u2 + 256]
                # V natural (per iter, M = 2 pairs' tokens)
                ppv = pqkp.tile([128, 256], F32, tag="pp",
                                name=f"pp_v_{ii}")
                nc.tensor.matmul(ppv[:, 0:128], xkT, W["wv_pi"],
                                 tile_position=(0, 0))
                v_sb = mid.tile([128, 128], BF16, tag="v_sb")
                if has_bv:
                    nc.vector.tensor_tensor(v_sb, ppv[:, 0:128],
                                            ROW["bv_pi_row"], AX.add)
                else:
                    nc.vector.tensor_copy(v_sb, ppv[:, 0:128])

                if stage < 2:
                    continue
                # ---- scores (PE): per strip g one S and one ST matmul ----
                # psg [128, 2048] (4 banks): per g: S@512g(256), ST@512g+256
                psg = psgp.tile([128, 2048], F32, tag="psg")
                for g in range(4):
                    st = slice(32 * g, 32 * g + 32)
                    nc.tensor.matmul(psg[:, 512 * g:512 * g + 256],
                                     qtpi[st, :], ktab[st, :],
                                     tile_position=(32 * g, 0))
                    nc.tensor.matmul(psg[:, 512 * g + 256:512 * g + 512],
                                     ktpi[st, :], qtab[st, :],
                                     tile_position=(32 * g, 0))

                if stage < 3:
                    continue
                # ---- S -> SBUF fp32; ST -> SBUF bf16 (valid quadrants) ----
                if i % 4 == 0:
                    s_sb2 = bigp.tile([128, 2048], F32, tag="s_sb2")
                s_sb = s_sb2[:, 512 * (i % 4):512 * (i % 4) + 512]
                st_sb = bigp.tile([128, 512], BF16, tag="st_sb")
                # s_sb col = 64h + k, h = 4b + g  ->  col = 256b + 64g + k
                s_v = s_sb.rearrange("P (b g k) -> P g b k", b=2, g=4)
                # psg S col = 512g + 128r + 64b + k (r = rhs pair half)
                psg_s = psg.rearrange("P (g q r b k) -> P g q r b k",
                                      g=4, q=2, r=2, b=2)
                nc.vector.tensor_copy(s_v[0:64], psg_s[0:64, :, 0, 0, :, :])
                nc.scalar.copy(s_v[64:128], psg_s[64:128, :, 0, 1, :, :])
                # st_sb col = 128g + c (c: [hg 64 | hg4 64]), rows (p, kt)
                st_v = st_sb.rearrange("P (g c) -> P g c", g=4)
                psg_t = psg.rearrange("P (g q r c) -> P g q r c",
                                      g=4, q=2, r=2)
                nc.vector.tensor_copy(st_v[0:64], psg_t[0:64, :, 1, 0, :])
                nc.scalar.copy(st_v[64:128], psg_t[64:128, :, 1, 1, :])

                if i % 4 == 3:
                    nc.sync.dma_start(out=s_out[i // 4], in_=s_sb2)

                if stage < 4:
                    continue
                # ---- value matmuls (PE): K=64 per (pair, strip) ----
                # ptv [128, 1024] (2 banks): TVCT_p0@0:256, ptg@256:384,
                #                            TVCT_p1@512:768
                ptv = ptvp.tile([128, 1024], F32, tag="ptv")
                for p in range(2):
                    pk = slice(64 * p, 64 * p + 64)
                    for g in range(4):
                        st = slice(32 * g, 32 * g + 32)
                        rhs = st_sb[pk, 128 * g:128 * g + 128]
                        nc.tensor.matmul(ptv[st, 512 * p:512 * p + 128],
                                         v_sb[pk, st], rhs,
                                         tile_position=(64 * p, 32 * g))
                        nc.tensor.matmul(ptv[st, 512 * p + 128:512 * p + 256],
                                         xtp[pk, st], rhs,
                                         tile_position=(64 * p, 32 * g))
                tvct = mid.tile([128, 512], BF16, tag="tvct")
                nc.vector.tensor_copy(tvct[:, 0:256], ptv[:, 0:256])
                nc.scalar.copy(tvct[:, 256:512], ptv[:, 512:768])

                if stage < 5:
                    continue
                # ---- output projections + residual (PE) ----
                pot = ptv[:, 768:896]
                ptg = ptv[:, 256:384]
                for p in range(2):
                    tok = slice(64 * p, 64 * p + 64)
                    c = 256 * p
                    if has_bot:
                        nc.tensor.matmul(pot[tok, :], ones_col[:, 0:64],
                                         ROW["bot_row"], start=True,
                                         stop=False, tile_position=(0, 64 * p))
                    nc.tensor.matmul(pot[tok, :], tvct[:, c:c + 64],
                                     W["wot_a"], start=not has_bot,
                                     stop=False, tile_position=(0, 64 * p))
                    nc.tensor.matmul(pot[tok, :], tvct[:, c + 64:c + 128],
                                     W["wot_b"], start=False, stop=False,
                                     tile_position=(0, 64 * p))
                    nc.tensor.matmul(pot[tok, :], xqT[:, tok], W["ident"],
                                     start=False, stop=True,
                                     tile_position=(0, 64 * p))
                    if has_btg:
                        nc.tensor.matmul(ptg[tok, :], ones_col[:, 0:64],
                                         ROW["btg_row"], start=True,
                                         stop=False, tile_position=(0, 64 * p))
                    nc.tensor.matmul(ptg[tok, :], tvct[:, c + 128:c + 192],
                                     W["wtg_a"], start=not has_btg,
                                     stop=False, tile_position=(0, 64 * p))
                    nc.tensor.matmul(ptg[tok, :], tvct[:, c + 192:c + 256],
                                     W["wtg_b"], start=False, stop=True,
                                     tile_position=(0, 64 * p))

                if stage < 6:
                    continue
                # ---- LayerNorms ----
                for (src, outdram, gamma, beta, use_gb, on_act) in (
                        (pot, ot_out, "g_time_row", "b_time_row", gb_time,
                         True),
                        (ptg, tg_out, "g_tgt_row", "b_tgt_row", gb_tgt,
                         False)):
                    st6 = statp.tile([128, 6], F32, tag="st6")
                    mv = statp.tile([128, 2], F32, tag="mv")
                    rstd = statp.tile([128, 1], F32, tag="rstd")
                    nc.vector.bn_stats(st6, src[:, :])
                    nc.vector.bn_aggr(mv, st6)
                    nc.scalar.activation(rstd, mv[:, 1:2], AF.Sqrt,
                                         bias=eps_t)
                    nc.vector.reciprocal(rstd, rstd)
                    if i % 4 == 0:
                        o4 = outp.tile([128, 512], F32, tag="o4_" + gamma,
                                       name=f"o4_{gamma}_{ii}")
                        O4[gamma] = o4
                    o_sb = O4[gamma][:, 128 * (i % 4):128 * (i % 4) + 128]
                    if not use_gb:
                        if on_act:
                            nmu = statp.tile([128, 1], F32, tag="nmu")
                            nc.vector.tensor_scalar(nmu, mv[:, 0:1], rstd,
                                                    -1.0, AX.mult, AX.mult)
                            nc.scalar.activation(o_sb, src[:, :], AF.Identity,
                                                 bias=nmu, scale=rstd)
                        else:
                            nc.vector.tensor_scalar(o_sb, src[:, :],
                                                    mv[:, 0:1], rstd,
                                                    AX.subtract, AX.mult)
                    else:
                        z = outp.tile([128, 128], F32, tag="z_" + gamma)
                        nc.vector.scalar_tensor_tensor(
                            z, src[:, :], mv[:, 0:1], ROW[gamma],
                            AX.subtract, AX.mult)
                        nc.vector.scalar_tensor_tensor(
                            o_sb, z, rstd, ROW[beta],
                            AX.mult, AX.add)
                    if i % 4 == 3:
                        nc.sync.dma_start(out=outdram[i // 4], in_=O4[gamma])

    nc.compile()
    return nc


_BUILD_CACHE = {}


def _get_nc(key, **kw):
    if key not in _BUILD_CACHE:
        _BUILD_CACHE[key] = build_bass(**kw)
    return _BUILD_CACHE[key]


def kernel(time_features_Q, time_features_K, target_features_K,
           Wq, bq, Wk, bk, Wv, bv, Wot, bot, Wtg, btg,
           g_time, b_time, g_tgt, b_tgt):
    args = [np.asarray(a, dtype=np.float32) for a in
            (time_features_Q, time_features_K, target_features_K,
             Wq, bq, Wk, bk, Wv, bv, Wot, bot, Wtg, btg,
             g_time, b_time, g_tgt, b_tgt)]
    (Xq, Xk, Xt, Wq, bq, Wk, bk, Wv, bv, Wot, bot, Wtg, btg,
     g_time, b_time, g_tgt, b_tgt) = args

    wnp = _weights_np(Wq, bq, Wk, bk, Wv, bv, Wot, bot, Wtg, btg,
                      g_time, b_time, g_tgt, b_tgt)
    has_bv = bool(np.any(bv != 0))
    has_bot = bool(np.any(bot != 0))
    has_btg = bool(np.any(btg != 0))
    gb_time = bool(np.any(g_time != 1) or np.any(b_time != 0))
    gb_tgt = bool(np.any(g_tgt != 1) or np.any(b_tgt != 0))
    key = (ITERS, has_bv, has_bot, has_btg, gb_time, gb_tgt)
    nc = _get_nc(key, iters=ITERS, has_bv=has_bv, has_bot=has_bot,
                 has_btg=has_btg, gb_time=gb_time, gb_tgt=gb_tgt)

    def shard(x, perm=None):
        flat = x.reshape(BN, L, D)
        if perm is not None:
            flat = flat[:, :, perm]
        pad = np.concatenate([flat, np.zeros((BN_PAD - BN, L, D),
                                             np.float32)], axis=0)
        return pad.astype(bf16).reshape(N_CORES, ITERS, 128, D)

    xq_s, xk_s, xt_s = shard(Xq), shard(Xk), shard(Xt, perm=PI)

    in_maps = []
    for c in range(N_CORES):
        m = {"xq": np.ascontiguousarray(xq_s[c]),
             "xk": np.ascontiguousarray(xk_s[c]),
             "xt": np.ascontiguousarray(xt_s[c])}
        m.update({k: np.ascontiguousarray(v) for k, v in wnp.items()})
        in_maps.append(m)

    res = run_bass_kernel_spmd(nc, in_maps, core_ids=list(range(N_CORES)))
    outs = res.results

    def unstage_s(a):
        # [ITERS//4, 128, 2048] -> [pairs, h, q, k]
        a = a.reshape(ITERS // 4, 128, 4, 512).transpose(0, 2, 1, 3)
        return a.reshape(PAIRS_PER_CORE, 64, 8, 64).transpose(0, 2, 1, 3)

    def unstage_o(a):
        # [ITERS//4, 128, 512] -> [pairs, 64, 128]
        a = a.reshape(ITERS // 4, 128, 4, 128).transpose(0, 2, 1, 3)
        return a.reshape(PAIRS_PER_CORE, 64, 128)

    s_full = np.concatenate([unstage_s(r["s_out"]) for r in outs], axis=0)
    ot_full = np.concatenate([unstage_o(r["ot_out"]) for r in outs], axis=0)
    tg_full = np.concatenate([unstage_o(r["tg_out"]) for r in outs], axis=0)

    attn_score = s_full[:BN].reshape(B, NN, 8, 64, 64).astype(np.float32)
    time_value = ot_full[:BN].reshape(B, NN, 64, 128).astype(np.float32)
    cross_target = tg_full[:BN].reshape(B, NN, 64, 128).astype(np.float32)
    return attn_score, time_value, cross_target


# revision 22
# speedup vs baseline: 1.6609x; 1.1482x over previous
"""Trainium2 Bass kernel for nn_DecoderTimeAttention.

Per (batch, node) pair (828 total, padded to 832 = 8 cores x 104 pairs,
2 pairs per kernel iteration):
  Q = Xq @ Wq * 0.25, K = Xk @ Wk, V = Xk @ Wv       (per-head dk=16, H=8)
  S_h = Q_h K_h^T          -> attn_score output (fp32)
  tv  = concat_h(S_h V_h) @ Wot ; time_value = LN(tv + Xq)
  ct  = concat_h(S_h T_h) @ Wtg ; cross_target = LN(ct)

Layout: head dims permuted (PI) so strip g (rows 32g..32g+31) holds head g
(j<16) and head g+4 (j>=16).  Scores: one K=32 matmul per strip with N=256
spanning masked K/Q variants for both pairs (off-diagonal garbage quadrants
are skipped by the copies).  ST comes out pair-on-partitions; value matmuls
are K=64 per (pair, strip) with combined (head, head+4) rhs; junk rows are
killed by zero rows in the Wot/Wtg A/B variants.  Residual added on the PE
via an identity matmul.  One PSUM bank per score strip (concurrent row-tiles
must not share a bank).
"""

import numpy as np
import ml_dtypes

import concourse.bass as bass
import concourse.bacc as bacc
import concourse.tile as tile
from concourse import mybir
from concourse.bass_utils import run_bass_kernel_spmd

F32 = mybir.dt.float32
BF16 = mybir.dt.bfloat16
AX = mybir.AluOpType
AF = mybir.ActivationFunctionType

N_CORES = 8
B, NN, L, D = 4, 207, 64, 128
H, DK = 8, 16
BN = B * NN              # 828
BN_PAD = 832             # 8 * 104
PAIRS_PER_CORE = BN_PAD // N_CORES   # 104
ITERS = PAIRS_PER_CORE // 2          # 52
LN_EPS = 1e-5

bf16 = ml_dtypes.bfloat16


def _perm():
    # pi[32g + j] = 16g + j (j<16) else 16(g+4) + (j-16)
    p = np.zeros(128, dtype=np.int64)
    for g in range(4):
        for j in range(32):
            p[32 * g + j] = 16 * g + j if j < 16 else 16 * (g + 4) + (j - 16)
    return p


PI = _perm()
MASK_TOP = np.array([1.0 if (i % 32) < 16 else 0.0 for i in range(128)],
                    dtype=np.float32)
MASK_BOT = 1.0 - MASK_TOP


def _weights_np(Wq, bq, Wk, bk, Wv, bv, Wot, bot, Wtg, btg,
                g_time, b_time, g_tgt, b_tgt):
    """Host-side weight preprocessing. Returns dict name->np array."""
    s = 0.25  # 1/sqrt(dk)
    wq_pi = (Wq * s)[:, PI]
    wk_pi = Wk[:, PI]
    wv_pi = Wv[:, PI]
    out = {
        "wq_pi": wq_pi,
        "wq_top": wq_pi * MASK_TOP[None, :],
        "wq_bot": wq_pi * MASK_BOT[None, :],
        "wk_pi": wk_pi,
        "wk_top": wk_pi * MASK_TOP[None, :],
        "wk_bot": wk_pi * MASK_BOT[None, :],
        "wv_pi": wv_pi,
        "wot_a": Wot[PI, :] * MASK_TOP[:, None],
        "wot_b": Wot[PI, :] * MASK_BOT[:, None],
        "wtg_a": Wtg[PI, :] * MASK_TOP[:, None],
        "wtg_b": Wtg[PI, :] * MASK_BOT[:, None],
        "ident": np.eye(128, dtype=np.float32),
    }
    out = {k: v.astype(bf16) for k, v in out.items()}
    worder = ["wq_pi", "wq_top", "wq_bot", "wk_pi", "wk_top", "wk_bot",
              "wv_pi", "wot_a", "wot_b", "wtg_a", "wtg_b", "ident"]
    out = {"wstack": np.concatenate([out[n] for n in worder], axis=1)}
    # fp32 per-partition biases for the Q/K casts (zero-cost in tensor_scalar)
    bq_pi = (bq * s)[PI].astype(np.float32)
    bk_pi = bk[PI].astype(np.float32)
    out["bstack"] = np.stack(
        [bq_pi, bq_pi * MASK_TOP, bq_pi * MASK_BOT,
         bk_pi, bk_pi * MASK_TOP, bk_pi * MASK_BOT], axis=1)
    # conditional extras (fp32 rows, DMA-broadcast to 128 partitions)
    out["rstack"] = np.stack([bot, btg], axis=0).astype(bf16)
    out["bv_pi_row"] = bv[PI][None, :].astype(np.float32)
    out["g_time_row"] = g_time[None, :].astype(np.float32)
    out["b_time_row"] = b_time[None, :].astype(np.float32)
    out["g_tgt_row"] = g_tgt[None, :].astype(np.float32)
    out["b_tgt_row"] = b_tgt[None, :].astype(np.float32)
    return out


def build_bass(iters=ITERS, has_bv=False, has_bot=False, has_btg=False,
               gb_time=False, gb_tgt=False, stage=99, repeat=1):
    """Build the Bass module (one NeuronCore program, SPMD across 8)."""
    nc = bacc.Bacc("TRN2", target_bir_lowering=False, debug=False,
                   enable_asserts=False)

    xq = nc.dram_tensor("xq", [iters, 128, 128], BF16, kind="ExternalInput")
    xk = nc.dram_tensor("xk", [iters, 128, 128], BF16, kind="ExternalInput")
    xt = nc.dram_tensor("xt", [iters, 128, 128], BF16, kind="ExternalInput")
    wnames_bf = ["wq_pi", "wq_top", "wq_bot", "wk_pi", "wk_top", "wk_bot",
                 "wv_pi", "wot_a", "wot_b", "wtg_a", "wtg_b", "ident"]
    wstack_d = nc.dram_tensor("wstack", [128, 12 * 128], BF16,
                              kind="ExternalInput")
    bnames = ["bq_pi", "bq_top", "bq_bot", "bk_pi", "bk_top", "bk_bot"]
    bstack_d = nc.dram_tensor("bstack", [128, 6], F32, kind="ExternalInput")
    rows_bf = ["bot_row", "btg_row"]
    rstack_d = nc.dram_tensor("rstack", [2, 128], BF16,
                              kind="ExternalInput")
    rows_f32 = ["bv_pi_row", "g_time_row", "b_time_row", "g_tgt_row",
                "b_tgt_row"]
    gd = {n: nc.dram_tensor(n, [1, 128], F32, kind="ExternalInput")
          for n in rows_f32}

    s_out = nc.dram_tensor("s_out", [iters // 4, 128, 2048], F32,
                           kind="ExternalOutput")
    ot_out = nc.dram_tensor("ot_out", [iters // 4, 128, 512], F32,
                            kind="ExternalOutput")
    tg_out = nc.dram_tensor("tg_out", [iters // 4, 128, 512], F32,
                            kind="ExternalOutput")

    with tile.TileContext(nc) as tc:
        with (
            tc.tile_pool(name="consts", bufs=1) as consts,
            tc.tile_pool(name="inp", bufs=3) as inp,
            tc.tile_pool(name="mid", bufs=2) as mid,
            tc.tile_pool(name="big", bufs=2) as bigp,
            tc.tile_pool(name="outp", bufs=3) as outp,
            tc.tile_pool(name="stats", bufs=4) as statp,
            tc.tile_pool(name="pqk", bufs=2, space="PSUM") as pqkp,
            tc.tile_pool(name="psg", bufs=1, space="PSUM") as psgp,
            tc.tile_pool(name="ptv", bufs=1, space="PSUM") as ptvp,
        ):
            wstack_t = consts.tile([128, 12 * 128], BF16, tag="wstack")
            nc.sync.dma_start(out=wstack_t, in_=wstack_d[:, :])
            W = {n: wstack_t[:, 128 * j:128 * j + 128]
                 for j, n in enumerate(wnames_bf)}
            bstack_t = consts.tile([128, 6], F32, tag="bstack")
            nc.sync.dma_start(out=bstack_t, in_=bstack_d[:, :])
            BIA = {n: bstack_t[:, j:j + 1] for j, n in enumerate(bnames)}
            ROW = {}
            rstack_t = consts.tile([2, 128], BF16, tag="rstack")
            nc.sync.dma_start(out=rstack_t, in_=rstack_d[:, :])
            for j, n in enumerate(rows_bf):
                ROW[n] = rstack_t[j:j + 1, :]
            for n in rows_f32:
                ROW[n] = consts.tile([128, 128], F32, tag=n, name=n + "_sb")
                nc.sync.dma_start(out=ROW[n],
                                  in_=gd[n][0:1, :].to_broadcast([128, 128]))
            eps_t = consts.tile([128, 1], F32, tag="eps")
            nc.vector.memset(eps_t, LN_EPS)
            ones_col = consts.tile([1, 128], BF16, tag="ones_col")
            nc.vector.memset(ones_col, 1.0)

            O4 = {}
            for ii in range(iters * repeat):
                i = ii % iters
                # ---- input loads: 4-iter blocks; xq/xk transposed ----
                if i % 4 == 0:
                    xqT4 = inp.tile([128, 512], BF16, tag="xqT4")
                    nc.sync.dma_start(
                        out=xqT4,
                        in_=xq[i:i + 4].rearrange("c p d -> (c p) d"),
                        transpose=True)
                    xkT4 = inp.tile([128, 512], BF16, tag="xkT4")
                    nc.sync.dma_start(
                        out=xkT4,
                        in_=xk[i:i + 4].rearrange("c p d -> (c p) d"),
                        transpose=True)
                    xtp4 = inp.tile([128, 4, 128], BF16, tag="xtp4")
                    nc.sync.dma_start(
                        out=xtp4, in_=xt[i:i + 4].rearrange("c p d -> p c d"))
                xqT = xqT4[:, 128 * (i % 4):128 * (i % 4) + 128]
                xkT = xkT4[:, 128 * (i % 4):128 * (i % 4) + 128]
                xtp = xtp4[:, i % 4, :]

                # ---- projections per 2-iter block (4 pairs, N=256) ----
                if i % 2 == 0:
                    u4 = 256 * ((i % 4) // 2)
                    xqT2 = xqT4[:, u4:u4 + 256]
                    xkT2 = xkT4[:, u4:u4 + 256]
                    qtpi2 = mid.tile([128, 256], BF16, tag="qtpi2")
                    ktpi2 = mid.tile([128, 256], BF16, tag="ktpi2")
                    # qtab2/ktab2 cols: 4 pairs x [top | bot] (64 each)
                    qtab2 = mid.tile([128, 512], BF16, tag="qtab2")
                    ktab2 = mid.tile([128, 512], BF16, tag="ktab2")
                    qtab2_v = qtab2.rearrange("P (p v k) -> P p v k",
                                              p=4, v=2)
                    ktab2_v = ktab2.rearrange("P (p v k) -> P p v k",
                                              p=4, v=2)
                    for (wname, rhs2, outt, bia, on_act) in (
                            ("wq_pi", xqT2, qtpi2, "bq_pi", False),
                            ("wk_pi", xkT2, ktpi2, "bk_pi", True),
                            ("wq_top", xqT2, qtab2_v[:, :, 0, :], "bq_top",
                             False),
                            ("wq_bot", xqT2, qtab2_v[:, :, 1, :], "bq_bot",
                             True),
                            ("wk_top", xkT2, ktab2_v[:, :, 0, :], "bk_top",
                             False),
                            ("wk_bot", xkT2, ktab2_v[:, :, 1, :], "bk_bot",
                             True)):
                        pp = pqkp.tile([128, 256], F32, tag="pp",
                                       name=f"pp_{wname}_{ii}")
                        nc.tensor.matmul(pp, W[wname], rhs2,
                                         tile_position=(0, 0))
                        src_v = pp.rearrange("P (p k) -> P p k", p=4)
                        if on_act:
                            nc.scalar.activation(outt, src_v if outt is not
                                                 qtpi2 and outt is not ktpi2
                                                 else pp, AF.Identity,
                                                 bias=BIA[bia])
                        else:
                            nc.vector.tensor_scalar_add(
                                outt, src_v if outt is not qtpi2 and
                                outt is not ktpi2 else pp, BIA[bia])
                u2 = 128 * (i % 2)
                qtpi = qtpi2[:, u2:u2 + 128]
                ktpi = ktpi2[:, u2:u2 + 128]
                qtab = qtab2[:, 2 * u2:2 * u2 + 256]
                ktab = ktab2[:, 2 * u2:2 * u2 + 256]
                # V natural (per iter, M = 2 pairs' tokens)
                ppv = pqkp.tile([128, 256], F32, tag="pp",
                                name=f"pp_v_{ii}")
                nc.tensor.matmul(ppv[:, 0:128], xkT, W["wv_pi"],
                                 tile_position=(0, 0))
                v_sb = mid.tile([128, 128], BF16, tag="v_sb")
                if has_bv:
                    nc.vector.tensor_tensor(v_sb, ppv[:, 0:128],
                                            ROW["bv_pi_row"], AX.add)
                else:
                    nc.vector.tensor_copy(v_sb, ppv[:, 0:128])

                if stage < 2:
                    continue
                # ---- scores (PE): per strip g one S and one ST matmul ----
                # psg [128, 2048] (4 banks): per g: S@512g(256), ST@512g+256
                psg = psgp.tile([128, 2048], F32, tag="psg")
                for g in range(4):
                    st = slice(32 * g, 32 * g + 32)
                    nc.tensor.matmul(psg[:, 512 * g:512 * g + 256],
                                     qtpi[st, :], ktab[st, :],
                                     tile_position=(32 * g, 0))
                    nc.tensor.matmul(psg[:, 512 * g + 256:512 * g + 512],
                                     ktpi[st, :], qtab[st, :],
                                     tile_position=(32 * g, 0))

                if stage < 3:
                    continue
                # ---- S -> SBUF fp32; ST -> SBUF bf16 (valid quadrants) ----
                if i % 4 == 0:
                    s_sb2 = bigp.tile([128, 2048], F32, tag="s_sb2")
                s_sb = s_sb2[:, 512 * (i % 4):512 * (i % 4) + 512]
                st_sb = bigp.tile([128, 512], BF16, tag="st_sb")
                # s_sb col = 64h + k, h = 4b + g  ->  col = 256b + 64g + k
                s_v = s_sb.rearrange("P (b g k) -> P g b k", b=2, g=4)
                # psg S col = 512g + 128r + 64b + k (r = rhs pair half)
                psg_s = psg.rearrange("P (g q r b k) -> P g q r b k",
                                      g=4, q=2, r=2, b=2)
                nc.vector.tensor_copy(s_v[0:64], psg_s[0:64, :, 0, 0, :, :])
                nc.scalar.copy(s_v[64:128], psg_s[64:128, :, 0, 1, :, :])
                # st_sb col = 128g + c (c: [hg 64 | hg4 64]), rows (p, kt)
                st_v = st_sb.rearrange("P (g c) -> P g c", g=4)
                psg_t = psg.rearrange("P (g q r c) -> P g q r c",
                                      g=4, q=2, r=2)
                nc.vector.tensor_copy(st_v[0:64], psg_t[0:64, :, 1, 0, :])
                nc.scalar.copy(st_v[64:128], psg_t[64:128, :, 1, 1, :])

                if i % 4 == 3:
                    nc.sync.dma_start(out=s_out[i // 4], in_=s_sb2)

                if stage < 4:
                    continue
                # ---- value matmuls (PE): K=64 per (pair, strip) ----
                # ptv [128, 1024] (2 banks): TVCT_p0@0:256, ptg@256:384,
                #                            TVCT_p1@512:768
                ptv = ptvp.tile([128, 1024], F32, tag="ptv")
                for p in range(2):
                    pk = slice(64 * p, 64 * p + 64)
                    for g in range(4):
                        st = slice(32 * g, 32 * g + 32)
                        rhs = st_sb[pk, 128 * g:128 * g + 128]
                        nc.tensor.matmul(ptv[st, 512 * p:512 * p + 128],
                                         v_sb[pk, st], rhs,
                                         tile_position=(64 * p, 32 * g))
                        nc.tensor.matmul(ptv[st, 512 * p + 128:512 * p + 256],
                                         xtp[pk, st], rhs,
                                         tile_position=(64 * p, 32 * g))
                tvct = mid.tile([128, 512], BF16, tag="tvct")
                nc.vector.tensor_copy(tvct[:, 0:256], ptv[:, 0:256])
                nc.scalar.copy(tvct[:, 256:512], ptv[:, 512:768])

                if stage < 5:
                    continue
                # ---- output projections + residual (PE) ----
                pot = ptv[:, 768:896]
                ptg = ptv[:, 256:384]
                for p in range(2):
                    tok = slice(64 * p, 64 * p + 64)
                    c = 256 * p
                    if has_bot:
                        nc.tensor.matmul(pot[tok, :], ones_col[:, 0:64],
                                         ROW["bot_row"], start=True,
                                         stop=False, tile_position=(0, 64 * p))
                    nc.tensor.matmul(pot[tok, :], tvct[:, c:c + 64],
                                     W["wot_a"], start=not has_bot,
                                     stop=False, tile_position=(0, 64 * p))
                    nc.tensor.matmul(pot[tok, :], tvct[:, c + 64:c + 128],
                                     W["wot_b"], start=False, stop=False,
                                     tile_position=(0, 64 * p))
                    nc.tensor.matmul(pot[tok, :], xqT[:, tok], W["ident"],
                                     start=False, stop=True,
                                     tile_position=(0, 64 * p))
                    if has_btg:
                        nc.tensor.matmul(ptg[tok, :], ones_col[:, 0:64],
                                         ROW["btg_row"], start=True,
                                         stop=False, tile_position=(0, 64 * p))
                    nc.tensor.matmul(ptg[tok, :], tvct[:, c + 128:c + 192],
                                     W["wtg_a"], start=not has_btg,
                                     stop=False, tile_position=(0, 64 * p))
                    nc.tensor.matmul(ptg[tok, :], tvct[:, c + 192:c + 256],
                                     W["wtg_b"], start=False, stop=True,
                                     tile_position=(0, 64 * p))

                if stage < 6:
                    continue
                # ---- LayerNorms ----
                for (src, outdram, gamma, beta, use_gb, on_act) in (
                        (pot, ot_out, "g_time_row", "b_time_row", gb_time,
                         True),
                        (ptg, tg_out, "g_tgt_row", "b_tgt_row", gb_tgt,
                         False)):
                    st6 = statp.tile([128, 6], F32, tag="st6")
                    mv = statp.tile([128, 2], F32, tag="mv")
                    rstd = statp.tile([128, 1], F32, tag="rstd")
                    nc.vector.bn_stats(st6, src[:, :])
                    nc.vector.bn_aggr(mv, st6)
                    nc.scalar.activation(rstd, mv[:, 1:2], AF.Sqrt,
                                         bias=eps_t)
                    nc.vector.reciprocal(rstd, rstd)
                    if i % 4 == 0:
                        o4 = outp.tile([128, 512], F32, tag="o4_" + gamma,
                                       name=f"o4_{gamma}_{ii}")
                        O4[gamma] = o4
                    o_sb = O4[gamma][:, 128 * (i % 4):128 * (i % 4) + 128]
                    if not use_gb:
                        if on_act:
                            nmu = statp.tile([128, 1], F32, tag="nmu")
                            nc.vector.tensor_scalar(nmu, mv[:, 0:1], rstd,
                                                    -1.0, AX.mult, AX.mult)
                            nc.scalar.activation(o_sb, src[:, :], AF.Identity,
                                                 bias=nmu, scale=rstd)
                        else:
                            nc.vector.tensor_scalar(o_sb, src[:, :],
                                                    mv[:, 0:1], rstd,
                                                    AX.subtract, AX.mult)
                    else:
                        z = outp.tile([128, 128], F32, tag="z_" + gamma)
                        nc.vector.scalar_tensor_tensor(
                            z, src[:, :], mv[:, 0:1], ROW[gamma],
                            AX.subtract, AX.mult)
                        nc.vector.scalar_tensor_tensor(
                            o_sb, z, rstd, ROW[beta],
                            AX.mult, AX.add)
                    if i % 4 == 3:
                        nc.sync.dma_start(out=outdram[i // 4], in_=O4[gamma])

    nc.compile()
    return nc


_BUILD_CACHE = {}


def _get_nc(key, **kw):
    if key not in _BUILD_CACHE:
        _BUILD_CACHE[key] = build_bass(**kw)
    return _BUILD_CACHE[key]


def kernel(time_features_Q, time_features_K, target_features_K,
           Wq, bq, Wk, bk, Wv, bv, Wot, bot, Wtg, btg,
           g_time, b_time, g_tgt, b_tgt):
    args = [np.asarray(a, dtype=np.float32) for a in
            (time_features_Q, time_features_K, target_features_K,
             Wq, bq, Wk, bk, Wv, bv, Wot, bot, Wtg, btg,
             g_time, b_time, g_tgt, b_tgt)]
    (Xq, Xk, Xt, Wq, bq, Wk, bk, Wv, bv, Wot, bot, Wtg, btg,
     g_time, b_time, g_tgt, b_tgt) = args

    wnp = _weights_np(Wq, bq, Wk, bk, Wv, bv, Wot, bot, Wtg, btg,
                      g_time, b_time, g_tgt, b_tgt)
    has_bv = bool(np.any(bv != 0))
    has_bot = bool(np.any(bot != 0))
    has_btg = bool(np.any(btg != 0))
    gb_time = bool(np.any(g_time != 1) or np.any(b_time != 0))
    gb_tgt = bool(np.any(g_tgt != 1) or np.any(b_tgt != 0))
    key = (ITERS, has_bv, has_bot, has_btg, gb_time, gb_tgt)
    nc = _get_nc(key, iters=ITERS, has_bv=has_bv, has_bot=has_bot,
                 has_btg=has_btg, gb_time=gb_time, gb_tgt=gb_tgt)

    def shard(x, perm=None):
        flat = x.reshape(BN, L, D)
        if perm is not None:
            flat = flat[:, :, perm]
        pad = np.concatenate([flat, np.zeros((BN_PAD - BN, L, D),
                                             np.float32)], axis=0)
        return pad.astype(bf16).reshape(N_CORES, ITERS, 128, D)

    xq_s, xk_s, xt_s = shard(Xq), shard(Xk), shard(Xt, perm=PI)

    in_maps = []
    for c in range(N_CORES):
        m = {"xq": np.ascontiguousarray(xq_s[c]),
             "xk": np.ascontiguousarray(xk_s[c]),
             "xt": np.ascontiguousarray(xt_s[c])}
        m.update({k: np.ascontiguousarray(v) for k, v in wnp.items()})
        in_maps.append(m)

    res = run_bass_kernel_spmd(nc, in_maps, core_ids=list(range(N_CORES)))
    outs = res.results

    def unstage_s(a):
        # [ITERS//4, 128, 2048] -> [pairs, h, q, k]
        a = a.reshape(ITERS // 4, 128, 4, 512).transpose(0, 2, 1, 3)
        return a.reshape(PAIRS_PER_CORE, 64, 8, 64).transpose(0, 2, 1, 3)

    def unstage_o(a):
        # [ITERS//4, 128, 512] -> [pairs, 64, 128]
        a = a.reshape(ITERS // 4, 128, 4, 128).transpose(0, 2, 1, 3)
        return a.reshape(PAIRS_PER_CORE, 64, 128)

    s_full = np.concatenate([unstage_s(r["s_out"]) for r in outs], axis=0)
    ot_full = np.concatenate([unstage_o(r["ot_out"]) for r in outs], axis=0)
    tg_full = np.concatenate([unstage_o(r["tg_out"]) for r in outs], axis=0)

    attn_score = s_full[:BN].reshape(B, NN, 8, 64, 64).astype(np.float32)
    time_value = ot_full[:BN].reshape(B, NN, 64, 128).astype(np.float32)
    cross_target = tg_full[:BN].reshape(B, NN, 64, 128).astype(np.float32)
    return attn_score, time_value, cross_target
